# revision 24
# baseline (speedup 1.0000x reference)
"""Trainium2 Bass kernel for the ConstitutiveModel recurrence.

Math (per time step, batch B):
    stress_t, dW/dxi = grad free_energy(eps_t - eye, xi_t)
    xi_{t+1} = xi_t + DT * grad dissipation(-dW/dxi)

Implementation notes:
  * Pure data parallel over 8 cores (256 batch rows each, 2 chunks of 128).
  * Activations live transposed ([feature, batch]) so the stored [in, out]
    weights serve directly as matmul lhsT operands.
  * xi is never materialised: only its projection s = wW1[6:].T @ xi.T enters
    the free energy, and s evolves linearly: s += DT*(dW1.T @ wW1[6:]).T @ h1.
    s is accumulated in a persistent PSUM bank; the eps contribution is folded
    into the same bank via delta-eps matmuls, so z1 = psum_state every step.
  * All matmuls run in bf16 (4x PE rate vs fp32, and avoids the fp32 power
    throttle); PSUM accumulation stays fp32.  delta-eps is quantized with
    error feedback so the running state tracks eps exactly.  r1/s1 (the relu
    factors reused by the backward products) stay fp32 for accuracy.
  * g1 is streamed into a persistent [128, T*256] buffer; stress = w1out.T@g1
    is computed in 512-column batches every 2 steps and DMA'd straight from
    PSUM, removing per-step stress matmuls + copies from the loop.
"""

import numpy as np
import ml_dtypes

import bass_rust
import concourse.bass as bass
import concourse.tile as tile_mod
from concourse import mybir
from concourse.bass_utils import run_bass_kernel_spmd
from concourse.tile_scheduler import N_PROCS
from concourse.vector_clock import ScopedClock, VectorClock

B, T, NIV, H = 2048, 64, 10, 128
DT = 0.01
NCORES = 8
NPC = B // NCORES      # 256 batch rows per core
NCH = 2                # chunks per core
CN = NPC // NCH        # 128 = matmul free dim per chunk
F32 = mybir.dt.float32
BF16 = mybir.dt.bfloat16
BF = ml_dtypes.bfloat16

# ---------------------------------------------------------------------------
# Workarounds: this walrus build accepts at most ONE sync-wait per instruction.
# ---------------------------------------------------------------------------
_wsplit_ctr = [0]


def _split_multi_waits(nc):
    """Hoist all but one sem-wait of every instruction onto same-engine NoOps
    inserted immediately before it (engine queues consume instructions in
    block order, so the NoOps' waits complete before the instruction issues)."""
    for f in nc.m.functions:
        for bb in f.blocks:
            changed = False
            new_list = []
            for ins in bb.instructions:
                si = getattr(ins, "sync_info", None)
                if si is not None and si.on_wait is not None and len(si.on_wait) > 1:
                    changed = True
                    waits = list(si.on_wait)
                    # Keep the FIRST-added wait (the RAW producer) on the
                    # instruction; the hoisted NOPs then carry WAR/buffer-reuse
                    # waits that resolve early, so the chain-binding wait does
                    # not pay the extra NOP dispatch hop.
                    for w in waits[1:]:
                        nop = mybir.InstNoOp(name=f"WSPLIT-{_wsplit_ctr[0]}")
                        _wsplit_ctr[0] += 1
                        nop.engine = ins.engine
                        nop.sync_info = bass_rust.SyncInfo(on_wait=[w], on_update=[])
                        nc.register_instruction(nop, overwrite=True)
                        new_list.append(nop)
                    ins.sync_info = bass_rust.SyncInfo(
                        on_wait=[waits[0]], on_update=list(si.on_update)
                    )
                new_list.append(ins)
            if changed:
                bb.instructions = new_list


def _patched_drain_and_barrier(self, tick_clock, wait_clock):
    """The stock tail drain waits on every sem in the global clock at once;
    emit a chain of single-wait sync NOPs instead (SP queue is FIFO, so the
    drain itself needs no waits)."""
    nc = self.nc
    gc = tick_clock.global_clock
    for p in range(N_PROCS):
        if gc[p] == 0:
            continue
        single = [0] * N_PROCS
        single[p] = gc[p]
        nop = nc.sync.nop()
        wait_clock.add_sem_waits(nop.ins, ScopedClock({None: VectorClock(single)}))
    nc.sync.drain()
    nc.all_engine_barrier()
    assert self.sems is not None
    popped = nc._tile_sem_poison_stack.pop()
    assert popped is self._sem_poison
    nc.clear_and_free_semaphores(list(self.sems.allocated().values()))
    nc.all_engine_barrier()


tile_mod.TileContext._drain_and_barrier = _patched_drain_and_barrier

# ---------------------------------------------------------------------------
# Device program
# ---------------------------------------------------------------------------
_WEIGHT_SPECS = [
    ("w1eps", (6, H)),     # lhsT: z1 += w1eps.T @ delta_eps
    ("w2", (H, H)),        # lhsT: z2 = wW2.T @ a1
    ("w2bwd", (H, H)),     # lhsT: g1pre = (4*wW2*wW3).T... (fused backward)
    ("m1", (H, H)),        # lhsT: u1 = -(wW1xi.T dW1).T @ g1
    ("dw2", (H, H)),       # lhsT: u2 = dW2.T @ b1a
    ("d2bwd", (H, H)),     # lhsT: h1pre
    ("m2t", (H, H)),       # lhsT: s += DT*(dW1.T wW1xi).T @ h1
    ("w1out", (H, 6)),     # lhsT: stress = wW1[:6] @ g1
]
_BIAS_NAMES = ["wb1", "wb2", "db1", "db2"]

_CACHED_NC = None


def _build():
    nc = bass.Bass("TRN2", target_bir_lowering=False, debug=False, num_devices=NCORES)
    deps_d = nc.dram_tensor("deps", [6, T * 256], BF16, kind="ExternalInput")
    w_d = {n: nc.dram_tensor(n, list(s), BF16, kind="ExternalInput") for n, s in _WEIGHT_SPECS}
    b_d = {n: nc.dram_tensor(n, [H, 1], F32, kind="ExternalInput") for n in _BIAS_NAMES}
    out_d = nc.dram_tensor("stress", [6, T * 256], F32, kind="ExternalOutput")

    Relu = mybir.ActivationFunctionType.Relu
    ADD = mybir.AluOpType.add
    MAX = mybir.AluOpType.max
    MULT = mybir.AluOpType.mult

    with tile_mod.TileContext(nc) as tc:
        with tc.tile_pool(name="const", bufs=1) as cpool, \
             tc.tile_pool(name="sb", bufs=6) as sb, \
             tc.tile_pool(name="stps", bufs=1, space="PSUM") as stps, \
             tc.tile_pool(name="wkps", bufs=2, space="PSUM") as wkps, \
             tc.tile_pool(name="strps", bufs=1, space="PSUM") as strps:

            # DMA order: step-0 inputs first (deps group 0, first-layer weight,
            # bias), then the rest; later deps groups last (needed after 16 steps).
            w_s = {n: cpool.tile(list(s), BF16, name=f"w_{n}", tag=f"w_{n}")
                   for n, s in _WEIGHT_SPECS}
            b_s = {n: cpool.tile([H, 1], F32, name=f"b_{n}", tag=f"b_{n}")
                   for n in _BIAS_NAMES}
            deps_g = [cpool.tile([6, 4096], BF16, name=f"deps{g}", tag=f"deps{g}")
                      for g in range(4)]
            nc.sync.dma_start(out=deps_g[0][:, :], in_=deps_d[:, 0:4096])
            nc.sync.dma_start(out=w_s["w1eps"][:, :], in_=w_d["w1eps"][:, :])
            nc.sync.dma_start(out=b_s["wb1"][:, :], in_=b_d["wb1"][:, :])
            for n, _ in _WEIGHT_SPECS:
                if n != "w1eps":
                    nc.sync.dma_start(out=w_s[n][:, :], in_=w_d[n][:, :])
            for n in _BIAS_NAMES:
                if n != "wb1":
                    nc.sync.dma_start(out=b_s[n][:, :], in_=b_d[n][:, :])
            for g in range(1, 4):
                nc.sync.dma_start(out=deps_g[g][:, :], in_=deps_d[:, g * 4096:(g + 1) * 4096])
            # persistent g1 stream: stress input for the batched output matmuls
            G = cpool.tile([H, T * 256], BF16, name="gbuf", tag="gbuf")
            stg = cpool.tile([6, T * 256], F32, name="stg", tag="stg")

            state = [stps.tile([H, CN], F32, name=f"state{c}", tag=f"state{c}") for c in range(NCH)]

            cur = [{} for _ in range(NCH)]
            stress_pend = [None, None]
            NSTAGE = 15

            def emit_stage(t, c, s):
                st = state[c]
                d = cur[c]
                if s == 0:
                    grp = t // 16
                    col = 256 * (t % 16) + CN * c
                    ep_sl = deps_g[grp][:, col:col + CN]
                    # z1 (unbiased) accumulates in the persistent state bank
                    nc.tensor.matmul(st[:, :], w_s["w1eps"][:, :], ep_sl,
                                     start=(t == 0), stop=(t == T - 1),
                                     skip_group_check=True)
                elif s == 1:
                    d["r1"] = sb.tile([H, CN], F32, name=f"r1_{c}", tag=f"r1_{c}")
                    nc.scalar.activation(d["r1"][:, :], st[:, :], Relu, bias=b_s["wb1"][:, :])
                    # drain the 2-step-old stress copy on the Vector queue here:
                    # DVE idles while a1 waits for r1, so the copy rides for free
                    if t % 2 == 1 and stress_pend[c] is not None:
                        ps_old, scol_old = stress_pend[c]
                        nc.vector.tensor_copy(out=stg[:, scol_old:scol_old + 256],
                                              in_=ps_old[:, :])
                        stress_pend[c] = None
                elif s == 2:
                    d["a1"] = sb.tile([H, CN], BF16, name=f"a1_{c}", tag=f"a1_{c}")
                    nc.vector.tensor_tensor(d["a1"][:, :], d["r1"][:, :], d["r1"][:, :], MULT)
                elif s == 3:
                    # Batched stress rides the z2-wait bubble: the z2 matmul waits
                    # ~770ns for r1->a1, so a stress matmul placed BEFORE it in the
                    # PE FIFO executes for free.  Its PSUM->SBUF copy is deferred
                    # two steps so it never waits on the matmul (Scalar has slack).
                    if t % 2 == 1 and t >= 3:
                        scol = (t - 3) * 256 + c * 256
                        ps_str = strps.tile([6, 256], F32, name=f"str{c}", tag=f"str{c}")
                        nc.tensor.matmul(ps_str[:, :], w_s["w1out"][:, :],
                                         G[:, scol:scol + 256], start=True, stop=True)
                        stress_pend[c] = (ps_str, scol)
                    d["ps_z2"] = wkps.tile([H, CN], F32, name=f"psz2_{c}", tag=f"wk_{c}")
                    nc.tensor.matmul(d["ps_z2"][:, :], w_s["w2"][:, :], d["a1"][:, :],
                                     start=True, stop=True)
                elif s == 4:
                    d["r2"] = sb.tile([H, CN], BF16, name=f"r2_{c}", tag=f"r2_{c}")
                    nc.scalar.activation(d["r2"][:, :], d["ps_z2"][:, :], Relu, bias=b_s["wb2"][:, :])
                elif s == 5:
                    d["ps_g1"] = wkps.tile([H, CN], F32, name=f"psg1_{c}", tag=f"wk_{c}")
                    nc.tensor.matmul(d["ps_g1"][:, :], w_s["w2bwd"][:, :], d["r2"][:, :],
                                     start=True, stop=True)
                elif s == 6:
                    gcol = t * 256 + CN * c
                    d["g1"] = G[:, gcol:gcol + CN]
                    nc.vector.tensor_tensor(d["g1"], d["ps_g1"][:, :], d["r1"][:, :], MULT)
                elif s == 7:
                    d["ps_u1"] = wkps.tile([H, CN], F32, name=f"psu1_{c}", tag=f"wk_{c}")
                    nc.tensor.matmul(d["ps_u1"][:, :], w_s["m1"][:, :], d["g1"],
                                     start=True, stop=True)
                elif s == 8:
                    d["s1"] = sb.tile([H, CN], F32, name=f"s1_{c}", tag=f"s1_{c}")
                    nc.scalar.activation(d["s1"][:, :], d["ps_u1"][:, :], Relu, bias=b_s["db1"][:, :])
                elif s == 9:
                    d["b1a"] = sb.tile([H, CN], BF16, name=f"b1a_{c}", tag=f"b1a_{c}")
                    nc.vector.tensor_tensor(d["b1a"][:, :], d["s1"][:, :], d["s1"][:, :], MULT)
                elif s == 10:
                    d["ps_u2"] = wkps.tile([H, CN], F32, name=f"psu2_{c}", tag=f"wk_{c}")
                    nc.tensor.matmul(d["ps_u2"][:, :], w_s["dw2"][:, :], d["b1a"][:, :],
                                     start=True, stop=True)
                elif s == 11:
                    d["s2"] = sb.tile([H, CN], BF16, name=f"s2_{c}", tag=f"s2_{c}")
                    nc.scalar.activation(d["s2"][:, :], d["ps_u2"][:, :], Relu, bias=b_s["db2"][:, :])
                elif s == 12:
                    d["ps_h1"] = wkps.tile([H, CN], F32, name=f"psh1_{c}", tag=f"wk_{c}")
                    nc.tensor.matmul(d["ps_h1"][:, :], w_s["d2bwd"][:, :], d["s2"][:, :],
                                     start=True, stop=True)
                elif s == 13:
                    d["h1"] = sb.tile([H, CN], BF16, name=f"h1_{c}", tag=f"h1_{c}")
                    nc.vector.tensor_tensor(d["h1"][:, :], d["ps_h1"][:, :], d["s1"][:, :], MULT)
                elif s == 14:
                    if t < T - 1:
                        nc.tensor.matmul(st[:, :], w_s["m2t"][:, :], d["h1"][:, :],
                                         start=False, stop=False, skip_group_check=True)


            # software pipeline: chunk 1 trails chunk 0 by half a step, so each
            # engine queue's order matches the order operands become ready
            SKEW = NSTAGE // 2  # 7
            for t in range(T):
                for i in range(NSTAGE):
                    emit_stage(t, 0, i)
                    j = i - SKEW
                    if j >= 0:
                        emit_stage(t, 1, j)
                    elif t > 0:
                        emit_stage(t - 1, 1, j + NSTAGE)
            for j in range(NSTAGE - SKEW, NSTAGE):
                emit_stage(T - 1, 1, j)
            # drain pending copies (pair 60,61) and do the last pair (62,63)
            for c in range(NCH):
                if stress_pend[c] is not None:
                    ps_old, scol_old = stress_pend[c]
                    nc.scalar.activation(stg[:, scol_old:scol_old + 256], ps_old[:, :],
                                         mybir.ActivationFunctionType.Copy)
                    stress_pend[c] = None
            for k, scol in enumerate(range((T - 2) * 256, T * 256, 256)):
                ps_str = strps.tile([6, 256], F32, name="strf", tag=f"str{k % 2}")
                nc.tensor.matmul(ps_str[:, :], w_s["w1out"][:, :],
                                 G[:, scol:scol + 256], start=True, stop=True)
                nc.scalar.activation(stg[:, scol:scol + 256], ps_str[:, :],
                                     mybir.ActivationFunctionType.Copy)

            for g in range(4):
                nc.sync.dma_start(out=out_d[:, g * 4096:(g + 1) * 4096],
                                  in_=stg[:, g * 4096:(g + 1) * 4096])

    _split_multi_waits(nc)
    return nc


def _host_prep(inputs):
    f32 = np.float32
    wW1 = np.ascontiguousarray(inputs["wW1"], f32)
    wW2 = np.ascontiguousarray(inputs["wW2"], f32)
    wW3 = np.ascontiguousarray(inputs["wW3"], f32)
    dW1 = np.ascontiguousarray(inputs["dW1"], f32)
    dW2 = np.ascontiguousarray(inputs["dW2"], f32)
    dWc = np.ascontiguousarray(inputs["dWc"], f32)
    W1eps = wW1[:6]
    W1xi = wW1[6:]
    weights = {
        "w1eps": W1eps,
        "w2": wW2,
        "w2bwd": (wW2.T * (4.0 * wW3[:, 0])[:, None]),
        "m1": -(W1xi.T @ dW1),
        "dw2": dW2,
        "d2bwd": (dW2.T * (4.0 * dWc[:, 0] ** 2)[:, None]),
        "m2t": DT * (dW1.T @ W1xi),
        "w1out": W1eps.T,
    }
    weights = {n: np.ascontiguousarray(w.astype(f32).astype(BF)) for n, w in weights.items()}
    for n in _BIAS_NAMES:
        weights[n] = np.ascontiguousarray(inputs[n], f32).reshape(H, 1)
    return weights


def _pack_deps_all(eps):
    """eps [B,T,6] -> per-core delta-eps staging [NCORES][6, T*NPC] in bf16,
    quantized with error feedback so the cumsum of quantized deltas tracks
    (eps_t - eye) to within one bf16 ulp (no error accumulation in the
    recurrent state)."""
    eye = np.array([1.0, 0.0, 0.0, 1.0, 0.0, 1.0], np.float32)
    epsT = np.ascontiguousarray(eps.transpose(1, 2, 0))  # [T, 6, B]
    tgt = epsT.astype(np.float64)
    tgt -= eye[None, :, None]
    qd = np.zeros(epsT.shape, BF)
    run = np.zeros(epsT.shape[1:], np.float64)
    for t in range(T):
        qd[t] = (tgt[t] - run).astype(np.float32).astype(BF)
        run += qd[t].astype(np.float64)
    out = []
    for core in range(NCORES):
        blk = qd[:, :, core * NPC:(core + 1) * NPC]       # [T, 6, NPC]
        out.append(np.ascontiguousarray(blk.transpose(1, 0, 2).reshape(6, T * NPC)))
    return out


def _unpack_stress(S):
    """staging [6, T*256] -> [NPC, T, 6]."""
    return np.ascontiguousarray(S.reshape(6, T, NPC).transpose(2, 1, 0))


def kernel(**inputs):
    global _CACHED_NC
    if _CACHED_NC is None:
        _CACHED_NC = _build()
    nc = _CACHED_NC

    weights = _host_prep(inputs)
    eps = np.ascontiguousarray(inputs["eps"], np.float32)
    deps_cores = _pack_deps_all(eps)
    in_maps = []
    for core in range(NCORES):
        m = dict(weights)
        m["deps"] = deps_cores[core]
        in_maps.append(m)

    res = run_bass_kernel_spmd(nc, in_maps, core_ids=list(range(NCORES)))
    out = np.empty((B, T, 6), np.float32)
    for core in range(NCORES):
        out[core * NPC:(core + 1) * NPC] = _unpack_stress(res.results[core]["stress"])
    return out


# revision 28
# speedup vs baseline: 1.0482x; 1.0482x over previous
"""Trainium2 Bass kernel for the ConstitutiveModel recurrence.

Math (per time step, batch B):
    stress_t, dW/dxi = grad free_energy(eps_t - eye, xi_t)
    xi_{t+1} = xi_t + DT * grad dissipation(-dW/dxi)

Implementation notes:
  * Pure data parallel over 8 cores (256 batch rows each, 2 chunks of 128).
  * Activations live transposed ([feature, batch]) so the stored [in, out]
    weights serve directly as matmul lhsT operands.
  * xi is never materialised: only its projection s = wW1[6:].T @ xi.T enters
    the free energy, and s evolves linearly: s += DT*(dW1.T @ wW1[6:]).T @ h1.
    s is accumulated in a persistent PSUM bank; the eps contribution is folded
    into the same bank via delta-eps matmuls, so z1 = psum_state every step.
  * All matmuls run in bf16 (4x PE rate vs fp32, and avoids the fp32 power
    throttle); PSUM accumulation stays fp32.  delta-eps is quantized with
    error feedback so the running state tracks eps exactly.  r1/s1 (the relu
    factors reused by the backward products) stay fp32 for accuracy.
  * g1 is streamed into a persistent [128, T*256] buffer; stress = w1out.T@g1
    is computed in 512-column batches every 2 steps and DMA'd straight from
    PSUM, removing per-step stress matmuls + copies from the loop.
"""

import numpy as np
import ml_dtypes

import bass_rust
import concourse.bass as bass
import concourse.tile as tile_mod
from concourse import mybir
from concourse.bass_utils import run_bass_kernel_spmd
from concourse.tile_scheduler import N_PROCS
from concourse.vector_clock import ScopedClock, VectorClock

B, T, NIV, H = 2048, 64, 10, 128
DT = 0.01
NCORES = 8
NPC = B // NCORES      # 256 batch rows per core
NCH = 2                # chunks per core
CN = NPC // NCH        # 128 = matmul free dim per chunk
F32 = mybir.dt.float32
BF16 = mybir.dt.bfloat16
BF = ml_dtypes.bfloat16

# ---------------------------------------------------------------------------
# Workarounds: this walrus build accepts at most ONE sync-wait per instruction.
# ---------------------------------------------------------------------------
_wsplit_ctr = [0]


def _split_multi_waits(nc):
    """Hoist all but one sem-wait of every instruction onto same-engine NoOps
    inserted immediately before it (engine queues consume instructions in
    block order, so the NoOps' waits complete before the instruction issues)."""
    for f in nc.m.functions:
        for bb in f.blocks:
            changed = False
            new_list = []
            for ins in bb.instructions:
                si = getattr(ins, "sync_info", None)
                if si is not None and si.on_wait is not None and len(si.on_wait) > 1:
                    changed = True
                    waits = list(si.on_wait)
                    # Keep the FIRST-added wait (the RAW producer) on the
                    # instruction; the hoisted NOPs then carry WAR/buffer-reuse
                    # waits that resolve early, so the chain-binding wait does
                    # not pay the extra NOP dispatch hop.
                    for w in waits[1:]:
                        nop = mybir.InstNoOp(name=f"WSPLIT-{_wsplit_ctr[0]}")
                        _wsplit_ctr[0] += 1
                        nop.engine = ins.engine
                        nop.sync_info = bass_rust.SyncInfo(on_wait=[w], on_update=[])
                        nc.register_instruction(nop, overwrite=True)
                        new_list.append(nop)
                    ins.sync_info = bass_rust.SyncInfo(
                        on_wait=[waits[0]], on_update=list(si.on_update)
                    )
                new_list.append(ins)
            if changed:
                bb.instructions = new_list


def _patched_drain_and_barrier(self, tick_clock, wait_clock):
    """The stock tail drain waits on every sem in the global clock at once;
    emit a chain of single-wait sync NOPs instead (SP queue is FIFO, so the
    drain itself needs no waits)."""
    nc = self.nc
    gc = tick_clock.global_clock
    for p in range(N_PROCS):
        if gc[p] == 0:
            continue
        single = [0] * N_PROCS
        single[p] = gc[p]
        nop = nc.sync.nop()
        wait_clock.add_sem_waits(nop.ins, ScopedClock({None: VectorClock(single)}))
    nc.sync.drain()
    nc.all_engine_barrier()
    assert self.sems is not None
    popped = nc._tile_sem_poison_stack.pop()
    assert popped is self._sem_poison
    nc.clear_and_free_semaphores(list(self.sems.allocated().values()))
    nc.all_engine_barrier()


tile_mod.TileContext._drain_and_barrier = _patched_drain_and_barrier

# ---------------------------------------------------------------------------
# Device program
# ---------------------------------------------------------------------------
_WEIGHT_SPECS = [
    ("w1eps", (6, H)),     # lhsT: z1 += w1eps.T @ delta_eps
    ("w2", (H, H)),        # lhsT: z2 = wW2.T @ a1
    ("w2bwd", (H, H)),     # lhsT: g1pre = (4*wW2*wW3).T... (fused backward)
    ("m1", (H, H)),        # lhsT: u1 = -(wW1xi.T dW1).T @ g1
    ("dw2", (H, H)),       # lhsT: u2 = dW2.T @ b1a
    ("d2bwd", (H, H)),     # lhsT: h1pre
    ("m2t", (H, H)),       # lhsT: s += DT*(dW1.T wW1xi).T @ h1
    ("w1out", (H, 6)),     # lhsT: stress = wW1[:6] @ g1
]
_BIAS_NAMES = ["wb1", "wb2", "db1", "db2"]

_CACHED_NC = None


def _build():
    nc = bass.Bass("TRN2", target_bir_lowering=False, debug=False, num_devices=NCORES)
    deps_d = nc.dram_tensor("deps", [6, T * 256], BF16, kind="ExternalInput")
    w_d = {n: nc.dram_tensor(n, list(s), BF16, kind="ExternalInput") for n, s in _WEIGHT_SPECS}
    b_d = {n: nc.dram_tensor(n, [H, 1], F32, kind="ExternalInput") for n in _BIAS_NAMES}
    out_d = nc.dram_tensor("stress", [6, T * 256], F32, kind="ExternalOutput")

    Relu = mybir.ActivationFunctionType.Relu
    ADD = mybir.AluOpType.add
    MAX = mybir.AluOpType.max
    MULT = mybir.AluOpType.mult

    with tile_mod.TileContext(nc) as tc:
        with tc.tile_pool(name="const", bufs=1) as cpool, \
             tc.tile_pool(name="sb", bufs=6) as sb, \
             tc.tile_pool(name="stps", bufs=1, space="PSUM") as stps, \
             tc.tile_pool(name="wkps", bufs=2, space="PSUM") as wkps, \
             tc.tile_pool(name="strps", bufs=1, space="PSUM") as strps:

            # DMA order: step-0 inputs first (deps group 0, first-layer weight,
            # bias), then the rest; later deps groups last (needed after 16 steps).
            w_s = {n: cpool.tile(list(s), BF16, name=f"w_{n}", tag=f"w_{n}")
                   for n, s in _WEIGHT_SPECS}
            b_s = {n: cpool.tile([H, 1], F32, name=f"b_{n}", tag=f"b_{n}")
                   for n in _BIAS_NAMES}
            deps_g = [cpool.tile([6, 4096], BF16, name=f"deps{g}", tag=f"deps{g}")
                      for g in range(4)]
            nc.sync.dma_start(out=deps_g[0][:, :], in_=deps_d[:, 0:4096])
            nc.sync.dma_start(out=w_s["w1eps"][:, :], in_=w_d["w1eps"][:, :])
            nc.sync.dma_start(out=b_s["wb1"][:, :], in_=b_d["wb1"][:, :])
            for n, _ in _WEIGHT_SPECS:
                if n != "w1eps":
                    nc.sync.dma_start(out=w_s[n][:, :], in_=w_d[n][:, :])
            for n in _BIAS_NAMES:
                if n != "wb1":
                    nc.sync.dma_start(out=b_s[n][:, :], in_=b_d[n][:, :])
            for g in range(1, 4):
                nc.sync.dma_start(out=deps_g[g][:, :], in_=deps_d[:, g * 4096:(g + 1) * 4096])
            # persistent g1 stream: stress input for the batched output matmuls
            G = cpool.tile([H, T * 256], BF16, name="gbuf", tag="gbuf")
            stg = cpool.tile([6, T * 256], F32, name="stg", tag="stg")

            state = [stps.tile([H, CN], F32, name=f"state{c}", tag=f"state{c}") for c in range(NCH)]

            cur = [{} for _ in range(NCH)]
            stress_pend = [None, None]
            NSTAGE = 15

            def emit_stage(t, c, s):
                st = state[c]
                d = cur[c]
                if s == 0:
                    grp = t // 16
                    col = 256 * (t % 16) + CN * c
                    ep_sl = deps_g[grp][:, col:col + CN]
                    # z1 (unbiased) accumulates in the persistent state bank
                    nc.tensor.matmul(st[:, :], w_s["w1eps"][:, :], ep_sl,
                                     start=(t == 0), stop=(t == T - 1),
                                     skip_group_check=True)
                elif s == 1:
                    # r1 on DVE: the a1 square that follows is also on DVE, so it
                    # issues back-to-back in the same queue without a sem hop
                    d["r1"] = sb.tile([H, CN], F32, name=f"r1_{c}", tag=f"r1_{c}")
                    nc.vector.tensor_scalar(d["r1"][:, :], st[:, :], b_s["wb1"][:, :], 0.0, ADD, MAX)

                elif s == 2:
                    d["a1"] = sb.tile([H, CN], BF16, name=f"a1_{c}", tag=f"a1_{c}")
                    nc.vector.tensor_tensor(d["a1"][:, :], d["r1"][:, :], d["r1"][:, :], MULT)
                elif s == 3:
                    # Batched stress rides the z2-wait bubble: the z2 matmul waits
                    # ~770ns for r1->a1, so a stress matmul placed BEFORE it in the
                    # PE FIFO executes for free.  Its PSUM->SBUF copy is deferred
                    # two steps so it never waits on the matmul (Scalar has slack).
                    if t % 2 == 1 and t >= 3:
                        if stress_pend[c] is not None:
                            ps_old, scol_old = stress_pend[c]
                            nc.scalar.activation(stg[:, scol_old:scol_old + 256],
                                                 ps_old[:, :],
                                                 mybir.ActivationFunctionType.Copy)
                            stress_pend[c] = None
                        scol = (t - 3) * 256 + c * 256
                        ps_str = strps.tile([6, 256], F32, name=f"str{c}", tag=f"str{c}")
                        nc.tensor.matmul(ps_str[:, :], w_s["w1out"][:, :],
                                         G[:, scol:scol + 256], start=True, stop=True)
                        stress_pend[c] = (ps_str, scol)
                    d["ps_z2"] = wkps.tile([H, CN], F32, name=f"psz2_{c}", tag=f"wk_{c}")
                    nc.tensor.matmul(d["ps_z2"][:, :], w_s["w2"][:, :], d["a1"][:, :],
                                     start=True, stop=True)
                elif s == 4:
                    d["r2"] = sb.tile([H, CN], BF16, name=f"r2_{c}", tag=f"r2_{c}")
                    nc.scalar.activation(d["r2"][:, :], d["ps_z2"][:, :], Relu, bias=b_s["wb2"][:, :])
                elif s == 5:
                    d["ps_g1"] = wkps.tile([H, CN], F32, name=f"psg1_{c}", tag=f"wk_{c}")
                    nc.tensor.matmul(d["ps_g1"][:, :], w_s["w2bwd"][:, :], d["r2"][:, :],
                                     start=True, stop=True)
                elif s == 6:
                    gcol = t * 256 + CN * c
                    d["g1"] = G[:, gcol:gcol + CN]
                    nc.vector.tensor_tensor(d["g1"], d["ps_g1"][:, :], d["r1"][:, :], MULT)
                elif s == 7:
                    d["ps_u1"] = wkps.tile([H, CN], F32, name=f"psu1_{c}", tag=f"wk_{c}")
                    nc.tensor.matmul(d["ps_u1"][:, :], w_s["m1"][:, :], d["g1"],
                                     start=True, stop=True)
                elif s == 8:
                    d["s1"] = sb.tile([H, CN], F32, name=f"s1_{c}", tag=f"s1_{c}")
                    nc.vector.tensor_scalar(d["s1"][:, :], d["ps_u1"][:, :], b_s["db1"][:, :], 0.0, ADD, MAX)
                elif s == 9:
                    d["b1a"] = sb.tile([H, CN], BF16, name=f"b1a_{c}", tag=f"b1a_{c}")
                    nc.vector.tensor_tensor(d["b1a"][:, :], d["s1"][:, :], d["s1"][:, :], MULT)
                elif s == 10:
                    d["ps_u2"] = wkps.tile([H, CN], F32, name=f"psu2_{c}", tag=f"wk_{c}")
                    nc.tensor.matmul(d["ps_u2"][:, :], w_s["dw2"][:, :], d["b1a"][:, :],
                                     start=True, stop=True)
                elif s == 11:
                    d["s2"] = sb.tile([H, CN], BF16, name=f"s2_{c}", tag=f"s2_{c}")
                    nc.scalar.activation(d["s2"][:, :], d["ps_u2"][:, :], Relu, bias=b_s["db2"][:, :])
                elif s == 12:
                    d["ps_h1"] = wkps.tile([H, CN], F32, name=f"psh1_{c}", tag=f"wk_{c}")
                    nc.tensor.matmul(d["ps_h1"][:, :], w_s["d2bwd"][:, :], d["s2"][:, :],
                                     start=True, stop=True)
                elif s == 13:
                    d["h1"] = sb.tile([H, CN], BF16, name=f"h1_{c}", tag=f"h1_{c}")
                    nc.vector.tensor_tensor(d["h1"][:, :], d["ps_h1"][:, :], d["s1"][:, :], MULT)
                elif s == 14:
                    if t < T - 1:
                        nc.tensor.matmul(st[:, :], w_s["m2t"][:, :], d["h1"][:, :],
                                         start=False, stop=False, skip_group_check=True)


            # software pipeline: chunk 1 trails chunk 0 by half a step, so each
            # engine queue's order matches the order operands become ready
            SKEW = NSTAGE // 2  # 7
            for t in range(T):
                for i in range(NSTAGE):
                    emit_stage(t, 0, i)
                    j = i - SKEW
                    if j >= 0:
                        emit_stage(t, 1, j)
                    elif t > 0:
                        emit_stage(t - 1, 1, j + NSTAGE)
            for j in range(NSTAGE - SKEW, NSTAGE):
                emit_stage(T - 1, 1, j)
            # drain pending copies (pair 60,61) and do the last pair (62,63)
            for c in range(NCH):
                if stress_pend[c] is not None:
                    ps_old, scol_old = stress_pend[c]
                    nc.scalar.activation(stg[:, scol_old:scol_old + 256], ps_old[:, :],
                                         mybir.ActivationFunctionType.Copy)
                    stress_pend[c] = None
            for k, scol in enumerate(range((T - 2) * 256, T * 256, 256)):
                ps_str = strps.tile([6, 256], F32, name="strf", tag=f"str{k % 2}")
                nc.tensor.matmul(ps_str[:, :], w_s["w1out"][:, :],
                                 G[:, scol:scol + 256], start=True, stop=True)
                nc.scalar.activation(stg[:, scol:scol + 256], ps_str[:, :],
                                     mybir.ActivationFunctionType.Copy)

            for g in range(4):
                nc.sync.dma_start(out=out_d[:, g * 4096:(g + 1) * 4096],
                                  in_=stg[:, g * 4096:(g + 1) * 4096])

    _split_multi_waits(nc)
    return nc


def _host_prep(inputs):
    f32 = np.float32
    wW1 = np.ascontiguousarray(inputs["wW1"], f32)
    wW2 = np.ascontiguousarray(inputs["wW2"], f32)
    wW3 = np.ascontiguousarray(inputs["wW3"], f32)
    dW1 = np.ascontiguousarray(inputs["dW1"], f32)
    dW2 = np.ascontiguousarray(inputs["dW2"], f32)
    dWc = np.ascontiguousarray(inputs["dWc"], f32)
    W1eps = wW1[:6]
    W1xi = wW1[6:]
    weights = {
        "w1eps": W1eps,
        "w2": wW2,
        "w2bwd": (wW2.T * (4.0 * wW3[:, 0])[:, None]),
        "m1": -(W1xi.T @ dW1),
        "dw2": dW2,
        "d2bwd": (dW2.T * (4.0 * dWc[:, 0] ** 2)[:, None]),
        "m2t": DT * (dW1.T @ W1xi),
        "w1out": W1eps.T,
    }
    weights = {n: np.ascontiguousarray(w.astype(f32).astype(BF)) for n, w in weights.items()}
    for n in _BIAS_NAMES:
        weights[n] = np.ascontiguousarray(inputs[n], f32).reshape(H, 1)
    return weights


def _pack_deps_all(eps):
    """eps [B,T,6] -> per-core delta-eps staging [NCORES][6, T*NPC] in bf16,
    quantized with error feedback so the cumsum of quantized deltas tracks
    (eps_t - eye) to within one bf16 ulp (no error accumulation in the
    recurrent state)."""
    eye = np.array([1.0, 0.0, 0.0, 1.0, 0.0, 1.0], np.float32)
    epsT = np.ascontiguousarray(eps.transpose(1, 2, 0))  # [T, 6, B]
    tgt = epsT.astype(np.float64)
    tgt -= eye[None, :, None]
    qd = np.zeros(epsT.shape, BF)
    run = np.zeros(epsT.shape[1:], np.float64)
    for t in range(T):
        qd[t] = (tgt[t] - run).astype(np.float32).astype(BF)
        run += qd[t].astype(np.float64)
    out = []
    for core in range(NCORES):
        blk = qd[:, :, core * NPC:(core + 1) * NPC]       # [T, 6, NPC]
        out.append(np.ascontiguousarray(blk.transpose(1, 0, 2).reshape(6, T * NPC)))
    return out


def _unpack_stress(S):
    """staging [6, T*256] -> [NPC, T, 6]."""
    return np.ascontiguousarray(S.reshape(6, T, NPC).transpose(2, 1, 0))


def kernel(**inputs):
    global _CACHED_NC
    if _CACHED_NC is None:
        _CACHED_NC = _build()
    nc = _CACHED_NC

    weights = _host_prep(inputs)
    eps = np.ascontiguousarray(inputs["eps"], np.float32)
    deps_cores = _pack_deps_all(eps)
    in_maps = []
    for core in range(NCORES):
        m = dict(weights)
        m["deps"] = deps_cores[core]
        in_maps.append(m)

    res = run_bass_kernel_spmd(nc, in_maps, core_ids=list(range(NCORES)))
    out = np.empty((B, T, 6), np.float32)
    for core in range(NCORES):
        out[core * NPC:(core + 1) * NPC] = _unpack_stress(res.results[core]["stress"])
    return out


# revision 30
# speedup vs baseline: 1.0891x; 1.0390x over previous
"""Trainium2 Bass kernel for the ConstitutiveModel recurrence.

Math (per time step, batch B):
    stress_t, dW/dxi = grad free_energy(eps_t - eye, xi_t)
    xi_{t+1} = xi_t + DT * grad dissipation(-dW/dxi)

Implementation notes:
  * Pure data parallel over 8 cores (256 batch rows each, 2 chunks of 128).
  * Activations live transposed ([feature, batch]) so the stored [in, out]
    weights serve directly as matmul lhsT operands.
  * xi is never materialised: only its projection s = wW1[6:].T @ xi.T enters
    the free energy, and s evolves linearly: s += DT*(dW1.T @ wW1[6:]).T @ h1.
    s is accumulated in a persistent PSUM bank; the eps contribution is folded
    into the same bank via delta-eps matmuls, so z1 = psum_state every step.
  * All matmuls run in bf16 (4x PE rate vs fp32, and avoids the fp32 power
    throttle); PSUM accumulation stays fp32.  delta-eps is quantized with
    error feedback so the running state tracks eps exactly.  r1/s1 (the relu
    factors reused by the backward products) stay fp32 for accuracy.
  * g1 is streamed into a persistent [128, T*256] buffer; stress = w1out.T@g1
    is computed in 512-column batches every 2 steps and DMA'd straight from
    PSUM, removing per-step stress matmuls + copies from the loop.
"""

import numpy as np
import ml_dtypes

import bass_rust
import concourse.bass as bass
import concourse.tile as tile_mod
from concourse import mybir
from concourse.bass_utils import run_bass_kernel_spmd
from concourse.tile_scheduler import N_PROCS
from concourse.vector_clock import ScopedClock, VectorClock

B, T, NIV, H = 2048, 64, 10, 128
DT = 0.01
NCORES = 8
NPC = B // NCORES      # 256 batch rows per core
NCH = 2                # chunks per core
CN = NPC // NCH        # 128 = matmul free dim per chunk
F32 = mybir.dt.float32
BF16 = mybir.dt.bfloat16
BF = ml_dtypes.bfloat16

# ---------------------------------------------------------------------------
# Workarounds: this walrus build accepts at most ONE sync-wait per instruction.
# ---------------------------------------------------------------------------
_wsplit_ctr = [0]


def _split_multi_waits(nc):
    """Hoist all but one sem-wait of every instruction onto same-engine NoOps
    inserted immediately before it (engine queues consume instructions in
    block order, so the NoOps' waits complete before the instruction issues)."""
    for f in nc.m.functions:
        for bb in f.blocks:
            changed = False
            new_list = []
            for ins in bb.instructions:
                si = getattr(ins, "sync_info", None)
                if si is not None and si.on_wait is not None and len(si.on_wait) > 1:
                    changed = True
                    waits = list(si.on_wait)
                    # Keep the FIRST-added wait (the RAW producer) on the
                    # instruction; the hoisted NOPs then carry WAR/buffer-reuse
                    # waits that resolve early, so the chain-binding wait does
                    # not pay the extra NOP dispatch hop.
                    for w in waits[1:]:
                        nop = mybir.InstNoOp(name=f"WSPLIT-{_wsplit_ctr[0]}")
                        _wsplit_ctr[0] += 1
                        nop.engine = ins.engine
                        nop.sync_info = bass_rust.SyncInfo(on_wait=[w], on_update=[])
                        nc.register_instruction(nop, overwrite=True)
                        new_list.append(nop)
                    ins.sync_info = bass_rust.SyncInfo(
                        on_wait=[waits[0]], on_update=list(si.on_update)
                    )
                new_list.append(ins)
            if changed:
                bb.instructions = new_list


def _patched_drain_and_barrier(self, tick_clock, wait_clock):
    """The stock tail drain waits on every sem in the global clock at once;
    emit a chain of single-wait sync NOPs instead (SP queue is FIFO, so the
    drain itself needs no waits)."""
    nc = self.nc
    gc = tick_clock.global_clock
    for p in range(N_PROCS):
        if gc[p] == 0:
            continue
        single = [0] * N_PROCS
        single[p] = gc[p]
        nop = nc.sync.nop()
        wait_clock.add_sem_waits(nop.ins, ScopedClock({None: VectorClock(single)}))
    nc.sync.drain()
    nc.all_engine_barrier()
    assert self.sems is not None
    popped = nc._tile_sem_poison_stack.pop()
    assert popped is self._sem_poison
    nc.clear_and_free_semaphores(list(self.sems.allocated().values()))
    nc.all_engine_barrier()


tile_mod.TileContext._drain_and_barrier = _patched_drain_and_barrier

# ---------------------------------------------------------------------------
# Device program
# ---------------------------------------------------------------------------
_WEIGHT_SPECS = [
    ("w1eps", (6, H)),     # lhsT: z1 += w1eps.T @ delta_eps
    ("w2", (H, H)),        # lhsT: z2 = wW2.T @ a1
    ("w2bwd", (H, H)),     # lhsT: g1pre = (4*wW2*wW3).T... (fused backward)
    ("m1", (H, H)),        # lhsT: u1 = -(wW1xi.T dW1).T @ g1
    ("dw2", (H, H)),       # lhsT: u2 = dW2.T @ b1a
    ("d2bwd", (H, H)),     # lhsT: h1pre
    ("m2t", (H, H)),       # lhsT: s += DT*(dW1.T wW1xi).T @ h1
    ("w1out", (H, 6)),     # lhsT: stress = wW1[:6] @ g1
]
_BIAS_NAMES = ["wb1", "wb2", "db1", "db2"]

_CACHED_NC = None


def _build():
    nc = bass.Bass("TRN2", target_bir_lowering=False, debug=False, num_devices=NCORES)
    deps_d = nc.dram_tensor("deps", [6, T * 256], BF16, kind="ExternalInput")
    w_d = {n: nc.dram_tensor(n, list(s), BF16, kind="ExternalInput") for n, s in _WEIGHT_SPECS}
    b_d = {n: nc.dram_tensor(n, [H, 1], F32, kind="ExternalInput") for n in _BIAS_NAMES}
    out_d = nc.dram_tensor("stress", [6, T * 256], F32, kind="ExternalOutput")

    Relu = mybir.ActivationFunctionType.Relu
    ADD = mybir.AluOpType.add
    MAX = mybir.AluOpType.max
    MULT = mybir.AluOpType.mult

    with tile_mod.TileContext(nc) as tc:
        with tc.tile_pool(name="const", bufs=1) as cpool, \
             tc.tile_pool(name="sb", bufs=6) as sb, \
             tc.tile_pool(name="stps", bufs=1, space="PSUM") as stps, \
             tc.tile_pool(name="wkps", bufs=2, space="PSUM") as wkps, \
             tc.tile_pool(name="strps", bufs=1, space="PSUM") as strps:

            # DMA order: step-0 inputs first (deps group 0, first-layer weight,
            # bias), then the rest; later deps groups last (needed after 16 steps).
            w_s = {n: cpool.tile(list(s), BF16, name=f"w_{n}", tag=f"w_{n}")
                   for n, s in _WEIGHT_SPECS}
            b_s = {n: cpool.tile([H, 1], F32, name=f"b_{n}", tag=f"b_{n}")
                   for n in _BIAS_NAMES}
            deps_g = [cpool.tile([6, 4096], BF16, name=f"deps{g}", tag=f"deps{g}")
                      for g in range(4)]
            nc.sync.dma_start(out=deps_g[0][:, :], in_=deps_d[:, 0:4096])
            nc.sync.dma_start(out=w_s["w1eps"][:, :], in_=w_d["w1eps"][:, :])
            nc.sync.dma_start(out=b_s["wb1"][:, :], in_=b_d["wb1"][:, :])
            for n, _ in _WEIGHT_SPECS:
                if n != "w1eps":
                    nc.sync.dma_start(out=w_s[n][:, :], in_=w_d[n][:, :])
            for n in _BIAS_NAMES:
                if n != "wb1":
                    nc.sync.dma_start(out=b_s[n][:, :], in_=b_d[n][:, :])
            for g in range(1, 4):
                nc.sync.dma_start(out=deps_g[g][:, :], in_=deps_d[:, g * 4096:(g + 1) * 4096])
            # persistent g1 stream: stress input for the batched output matmuls
            G = cpool.tile([H, T * 256], BF16, name="gbuf", tag="gbuf")
            stg = cpool.tile([6, T * 256], F32, name="stg", tag="stg")

            state = [stps.tile([H, CN], F32, name=f"state{c}", tag=f"state{c}") for c in range(NCH)]

            cur = [{} for _ in range(NCH)]
            stress_pend = [None, None]
            NSTAGE = 15

            def emit_stage(t, c, s):
                st = state[c]
                d = cur[c]
                if s == 0:
                    grp = t // 16
                    col = 256 * (t % 16) + CN * c
                    ep_sl = deps_g[grp][:, col:col + CN]
                    # z1 (unbiased) accumulates in the persistent state bank
                    nc.tensor.matmul(st[:, :], w_s["w1eps"][:, :], ep_sl,
                                     start=(t == 0), stop=(t == T - 1),
                                     skip_group_check=True)
                elif s == 1:
                    d["r1"] = sb.tile([H, CN], F32, name=f"r1_{c}", tag=f"r1_{c}")
                    nc.scalar.activation(d["r1"][:, :], st[:, :], Relu, bias=b_s["wb1"][:, :])

                elif s == 2:
                    d["a1"] = sb.tile([H, CN], BF16, name=f"a1_{c}", tag=f"a1_{c}")
                    nc.vector.tensor_tensor(d["a1"][:, :], d["r1"][:, :], d["r1"][:, :], MULT)
                elif s == 3:
                    # Batched stress rides the z2-wait bubble: the z2 matmul waits
                    # ~770ns for r1->a1, so a stress matmul placed BEFORE it in the
                    # PE FIFO executes for free.  Its PSUM->SBUF copy is deferred
                    # two steps so it never waits on the matmul (Scalar has slack).
                    if t % 2 == 1 and t >= 3:
                        if stress_pend[c] is not None:
                            ps_old, scol_old = stress_pend[c]
                            nc.scalar.activation(stg[:, scol_old:scol_old + 256],
                                                 ps_old[:, :],
                                                 mybir.ActivationFunctionType.Copy)
                            stress_pend[c] = None
                        scol = (t - 3) * 256 + c * 256
                        ps_str = strps.tile([6, 256], F32, name=f"str{c}", tag=f"str{c}")
                        nc.tensor.matmul(ps_str[:, :], w_s["w1out"][:, :],
                                         G[:, scol:scol + 256], start=True, stop=True)
                        stress_pend[c] = (ps_str, scol)
                    d["ps_z2"] = wkps.tile([H, CN], F32, name=f"psz2_{c}", tag=f"wk_{c}")
                    nc.tensor.matmul(d["ps_z2"][:, :], w_s["w2"][:, :], d["a1"][:, :],
                                     start=True, stop=True)
                elif s == 4:
                    d["r2"] = sb.tile([H, CN], BF16, name=f"r2_{c}", tag=f"r2_{c}")
                    nc.scalar.activation(d["r2"][:, :], d["ps_z2"][:, :], Relu, bias=b_s["wb2"][:, :])
                elif s == 5:
                    d["ps_g1"] = wkps.tile([H, CN], F32, name=f"psg1_{c}", tag=f"wk_{c}")
                    nc.tensor.matmul(d["ps_g1"][:, :], w_s["w2bwd"][:, :], d["r2"][:, :],
                                     start=True, stop=True)
                elif s == 6:
                    gcol = t * 256 + CN * c
                    d["g1"] = G[:, gcol:gcol + CN]
                    nc.vector.tensor_tensor(d["g1"], d["ps_g1"][:, :], d["r1"][:, :], MULT)
                elif s == 7:
                    d["ps_u1"] = wkps.tile([H, CN], F32, name=f"psu1_{c}", tag=f"wk_{c}")
                    nc.tensor.matmul(d["ps_u1"][:, :], w_s["m1"][:, :], d["g1"],
                                     start=True, stop=True)
                elif s == 8:
                    d["s1"] = sb.tile([H, CN], F32, name=f"s1_{c}", tag=f"s1_{c}")
                    nc.scalar.activation(d["s1"][:, :], d["ps_u1"][:, :], Relu, bias=b_s["db1"][:, :])
                elif s == 9:
                    d["b1a"] = sb.tile([H, CN], BF16, name=f"b1a_{c}", tag=f"b1a_{c}")
                    nc.vector.tensor_tensor(d["b1a"][:, :], d["s1"][:, :], d["s1"][:, :], MULT)
                elif s == 10:
                    d["ps_u2"] = wkps.tile([H, CN], F32, name=f"psu2_{c}", tag=f"wk_{c}")
                    nc.tensor.matmul(d["ps_u2"][:, :], w_s["dw2"][:, :], d["b1a"][:, :],
                                     start=True, stop=True)
                elif s == 11:
                    d["s2"] = sb.tile([H, CN], BF16, name=f"s2_{c}", tag=f"s2_{c}")
                    nc.scalar.activation(d["s2"][:, :], d["ps_u2"][:, :], Relu, bias=b_s["db2"][:, :])
                elif s == 12:
                    d["ps_h1"] = wkps.tile([H, CN], F32, name=f"psh1_{c}", tag=f"wk_{c}")
                    nc.tensor.matmul(d["ps_h1"][:, :], w_s["d2bwd"][:, :], d["s2"][:, :],
                                     start=True, stop=True)
                elif s == 13:
                    d["h1"] = sb.tile([H, CN], BF16, name=f"h1_{c}", tag=f"h1_{c}")
                    nc.vector.tensor_tensor(d["h1"][:, :], d["ps_h1"][:, :], d["s1"][:, :], MULT)
                elif s == 14:
                    if t < T - 1:
                        nc.tensor.matmul(st[:, :], w_s["m2t"][:, :], d["h1"][:, :],
                                         start=False, stop=False, skip_group_check=True)


            # software pipeline: chunk 1 trails chunk 0 by half a step, so each
            # engine queue's order matches the order operands become ready
            SKEW = NSTAGE // 2  # 7
            for t in range(T):
                for i in range(NSTAGE):
                    emit_stage(t, 0, i)
                    j = i - SKEW
                    if j >= 0:
                        emit_stage(t, 1, j)
                    elif t > 0:
                        emit_stage(t - 1, 1, j + NSTAGE)
            for j in range(NSTAGE - SKEW, NSTAGE):
                emit_stage(T - 1, 1, j)
            # drain pending copies (pair 60,61) and do the last pair (62,63)
            for c in range(NCH):
                if stress_pend[c] is not None:
                    ps_old, scol_old = stress_pend[c]
                    nc.scalar.activation(stg[:, scol_old:scol_old + 256], ps_old[:, :],
                                         mybir.ActivationFunctionType.Copy)
                    stress_pend[c] = None
            for k, scol in enumerate(range((T - 2) * 256, T * 256, 256)):
                ps_str = strps.tile([6, 256], F32, name="strf", tag=f"str{k % 2}")
                nc.tensor.matmul(ps_str[:, :], w_s["w1out"][:, :],
                                 G[:, scol:scol + 256], start=True, stop=True)
                nc.scalar.activation(stg[:, scol:scol + 256], ps_str[:, :],
                                     mybir.ActivationFunctionType.Copy)

            for g in range(4):
                nc.sync.dma_start(out=out_d[:, g * 4096:(g + 1) * 4096],
                                  in_=stg[:, g * 4096:(g + 1) * 4096])

    _split_multi_waits(nc)
    return nc


def _host_prep(inputs):
    f32 = np.float32
    wW1 = np.ascontiguousarray(inputs["wW1"], f32)
    wW2 = np.ascontiguousarray(inputs["wW2"], f32)
    wW3 = np.ascontiguousarray(inputs["wW3"], f32)
    dW1 = np.ascontiguousarray(inputs["dW1"], f32)
    dW2 = np.ascontiguousarray(inputs["dW2"], f32)
    dWc = np.ascontiguousarray(inputs["dWc"], f32)
    W1eps = wW1[:6]
    W1xi = wW1[6:]
    weights = {
        "w1eps": W1eps,
        "w2": wW2,
        "w2bwd": (wW2.T * (4.0 * wW3[:, 0])[:, None]),
        "m1": -(W1xi.T @ dW1),
        "dw2": dW2,
        "d2bwd": (dW2.T * (4.0 * dWc[:, 0] ** 2)[:, None]),
        "m2t": DT * (dW1.T @ W1xi),
        "w1out": W1eps.T,
    }
    weights = {n: np.ascontiguousarray(w.astype(f32).astype(BF)) for n, w in weights.items()}
    for n in _BIAS_NAMES:
        weights[n] = np.ascontiguousarray(inputs[n], f32).reshape(H, 1)
    return weights


def _pack_deps_all(eps):
    """eps [B,T,6] -> per-core delta-eps staging [NCORES][6, T*NPC] in bf16,
    quantized with error feedback so the cumsum of quantized deltas tracks
    (eps_t - eye) to within one bf16 ulp (no error accumulation in the
    recurrent state)."""
    eye = np.array([1.0, 0.0, 0.0, 1.0, 0.0, 1.0], np.float32)
    epsT = np.ascontiguousarray(eps.transpose(1, 2, 0))  # [T, 6, B]
    tgt = epsT.astype(np.float64)
    tgt -= eye[None, :, None]
    qd = np.zeros(epsT.shape, BF)
    run = np.zeros(epsT.shape[1:], np.float64)
    for t in range(T):
        qd[t] = (tgt[t] - run).astype(np.float32).astype(BF)
        run += qd[t].astype(np.float64)
    out = []
    for core in range(NCORES):
        blk = qd[:, :, core * NPC:(core + 1) * NPC]       # [T, 6, NPC]
        out.append(np.ascontiguousarray(blk.transpose(1, 0, 2).reshape(6, T * NPC)))
    return out


def _unpack_stress(S):
    """staging [6, T*256] -> [NPC, T, 6]."""
    return np.ascontiguousarray(S.reshape(6, T, NPC).transpose(2, 1, 0))


def kernel(**inputs):
    global _CACHED_NC
    if _CACHED_NC is None:
        _CACHED_NC = _build()
    nc = _CACHED_NC

    weights = _host_prep(inputs)
    eps = np.ascontiguousarray(inputs["eps"], np.float32)
    deps_cores = _pack_deps_all(eps)
    in_maps = []
    for core in range(NCORES):
        m = dict(weights)
        m["deps"] = deps_cores[core]
        in_maps.append(m)

    res = run_bass_kernel_spmd(nc, in_maps, core_ids=list(range(NCORES)))
    out = np.empty((B, T, 6), np.float32)
    for core in range(NCORES):
        out[core * NPC:(core + 1) * NPC] = _unpack_stress(res.results[core]["stress"])
    return out


# revision 31
# speedup vs baseline: 1.0955x; 1.0059x over previous
"""Trainium2 Bass kernel for the ConstitutiveModel recurrence.

Math (per time step, batch B):
    stress_t, dW/dxi = grad free_energy(eps_t - eye, xi_t)
    xi_{t+1} = xi_t + DT * grad dissipation(-dW/dxi)

Implementation notes:
  * Pure data parallel over 8 cores (256 batch rows each, 2 chunks of 128).
  * Activations live transposed ([feature, batch]) so the stored [in, out]
    weights serve directly as matmul lhsT operands.
  * xi is never materialised: only its projection s = wW1[6:].T @ xi.T enters
    the free energy, and s evolves linearly: s += DT*(dW1.T @ wW1[6:]).T @ h1.
    s is accumulated in a persistent PSUM bank; the eps contribution is folded
    into the same bank via delta-eps matmuls, so z1 = psum_state every step.
  * All matmuls run in bf16 (4x PE rate vs fp32, and avoids the fp32 power
    throttle); PSUM accumulation stays fp32.  delta-eps is quantized with
    error feedback so the running state tracks eps exactly.  r1/s1 (the relu
    factors reused by the backward products) stay fp32 for accuracy.
  * g1 is streamed into a persistent [128, T*256] buffer; stress = w1out.T@g1
    is computed in 512-column batches every 2 steps and DMA'd straight from
    PSUM, removing per-step stress matmuls + copies from the loop.
"""

import numpy as np
import ml_dtypes

import bass_rust
import concourse.bass as bass
import concourse.tile as tile_mod
from concourse import mybir
from concourse.bass_utils import run_bass_kernel_spmd
from concourse.tile_scheduler import N_PROCS
from concourse.vector_clock import ScopedClock, VectorClock

B, T, NIV, H = 2048, 64, 10, 128
DT = 0.01
NCORES = 8
NPC = B // NCORES      # 256 batch rows per core
NCH = 2                # chunks per core
CN = NPC // NCH        # 128 = matmul free dim per chunk
F32 = mybir.dt.float32
BF16 = mybir.dt.bfloat16
BF = ml_dtypes.bfloat16

# ---------------------------------------------------------------------------
# Workarounds: this walrus build accepts at most ONE sync-wait per instruction.
# ---------------------------------------------------------------------------
_wsplit_ctr = [0]


def _split_multi_waits(nc):
    """Hoist all but one sem-wait of every instruction onto same-engine NoOps
    inserted immediately before it (engine queues consume instructions in
    block order, so the NoOps' waits complete before the instruction issues)."""
    for f in nc.m.functions:
        for bb in f.blocks:
            changed = False
            new_list = []
            for ins in bb.instructions:
                si = getattr(ins, "sync_info", None)
                if si is not None and si.on_wait is not None and len(si.on_wait) > 1:
                    changed = True
                    waits = list(si.on_wait)
                    # Keep the FIRST-added wait (the RAW producer) on the
                    # instruction; the hoisted NOPs then carry WAR/buffer-reuse
                    # waits that resolve early, so the chain-binding wait does
                    # not pay the extra NOP dispatch hop.
                    for w in waits[1:]:
                        nop = mybir.InstNoOp(name=f"WSPLIT-{_wsplit_ctr[0]}")
                        _wsplit_ctr[0] += 1
                        nop.engine = ins.engine
                        nop.sync_info = bass_rust.SyncInfo(on_wait=[w], on_update=[])
                        nc.register_instruction(nop, overwrite=True)
                        new_list.append(nop)
                    ins.sync_info = bass_rust.SyncInfo(
                        on_wait=[waits[0]], on_update=list(si.on_update)
                    )
                new_list.append(ins)
            if changed:
                bb.instructions = new_list


def _patched_drain_and_barrier(self, tick_clock, wait_clock):
    """The stock tail drain waits on every sem in the global clock at once;
    emit a chain of single-wait sync NOPs instead (SP queue is FIFO, so the
    drain itself needs no waits)."""
    nc = self.nc
    gc = tick_clock.global_clock
    for p in range(N_PROCS):
        if gc[p] == 0:
            continue
        single = [0] * N_PROCS
        single[p] = gc[p]
        nop = nc.sync.nop()
        wait_clock.add_sem_waits(nop.ins, ScopedClock({None: VectorClock(single)}))
    nc.sync.drain()
    nc.all_engine_barrier()
    assert self.sems is not None
    popped = nc._tile_sem_poison_stack.pop()
    assert popped is self._sem_poison
    nc.clear_and_free_semaphores(list(self.sems.allocated().values()))
    nc.all_engine_barrier()


tile_mod.TileContext._drain_and_barrier = _patched_drain_and_barrier

# ---------------------------------------------------------------------------
# Device program
# ---------------------------------------------------------------------------
_WEIGHT_SPECS = [
    ("w1eps", (6, H)),     # lhsT: z1 += w1eps.T @ delta_eps
    ("w2", (H, H)),        # lhsT: z2 = wW2.T @ a1
    ("w2bwd", (H, H)),     # lhsT: g1pre = (4*wW2*wW3).T... (fused backward)
    ("m1", (H, H)),        # lhsT: u1 = -(wW1xi.T dW1).T @ g1
    ("dw2", (H, H)),       # lhsT: u2 = dW2.T @ b1a
    ("d2bwd", (H, H)),     # lhsT: h1pre
    ("m2t", (H, H)),       # lhsT: s += DT*(dW1.T wW1xi).T @ h1
    ("w1out", (H, 6)),     # lhsT: stress = wW1[:6] @ g1
]
_BIAS_NAMES = ["wb1", "wb2", "db1", "db2"]

_CACHED_NC = None


def _build():
    nc = bass.Bass("TRN2", target_bir_lowering=False, debug=False, num_devices=NCORES)
    deps_d = nc.dram_tensor("deps", [6, T * 256], BF16, kind="ExternalInput")
    w_d = {n: nc.dram_tensor(n, list(s), BF16, kind="ExternalInput") for n, s in _WEIGHT_SPECS}
    b_d = {n: nc.dram_tensor(n, [H, 1], F32, kind="ExternalInput") for n in _BIAS_NAMES}
    out_d = nc.dram_tensor("stress", [6, T * 256], F32, kind="ExternalOutput")

    Relu = mybir.ActivationFunctionType.Relu
    ADD = mybir.AluOpType.add
    MAX = mybir.AluOpType.max
    MULT = mybir.AluOpType.mult

    with tile_mod.TileContext(nc) as tc:
        with tc.tile_pool(name="const", bufs=1) as cpool, \
             tc.tile_pool(name="sb", bufs=6) as sb, \
             tc.tile_pool(name="stps", bufs=1, space="PSUM") as stps, \
             tc.tile_pool(name="wkps", bufs=2, space="PSUM") as wkps, \
             tc.tile_pool(name="strps", bufs=1, space="PSUM") as strps:

            # DMA order: step-0 inputs first (deps group 0, first-layer weight,
            # bias), then the rest; later deps groups last (needed after 16 steps).
            w_s = {n: cpool.tile(list(s), BF16, name=f"w_{n}", tag=f"w_{n}")
                   for n, s in _WEIGHT_SPECS}
            b_s = {n: cpool.tile([H, 1], F32, name=f"b_{n}", tag=f"b_{n}")
                   for n in _BIAS_NAMES}
            deps_g = [cpool.tile([6, 4096], BF16, name=f"deps{g}", tag=f"deps{g}")
                      for g in range(4)]
            nc.sync.dma_start(out=deps_g[0][:, :], in_=deps_d[:, 0:4096])
            nc.sync.dma_start(out=w_s["w1eps"][:, :], in_=w_d["w1eps"][:, :])
            nc.sync.dma_start(out=b_s["wb1"][:, :], in_=b_d["wb1"][:, :])
            for n, _ in _WEIGHT_SPECS:
                if n != "w1eps":
                    nc.sync.dma_start(out=w_s[n][:, :], in_=w_d[n][:, :])
            for n in _BIAS_NAMES:
                if n != "wb1":
                    nc.sync.dma_start(out=b_s[n][:, :], in_=b_d[n][:, :])
            for g in range(1, 4):
                nc.sync.dma_start(out=deps_g[g][:, :], in_=deps_d[:, g * 4096:(g + 1) * 4096])
            # persistent g1 stream: stress input for the batched output matmuls
            G = cpool.tile([H, T * 256], BF16, name="gbuf", tag="gbuf")
            stg = cpool.tile([6, T * 256], F32, name="stg", tag="stg")

            state = [stps.tile([H, CN], F32, name=f"state{c}", tag=f"state{c}") for c in range(NCH)]

            cur = [{} for _ in range(NCH)]
            stress_pend = [None, None]
            NSTAGE = 15

            def emit_stage(t, c, s):
                st = state[c]
                d = cur[c]
                if s == 0:
                    grp = t // 16
                    col = 256 * (t % 16) + CN * c
                    ep_sl = deps_g[grp][:, col:col + CN]
                    # z1 (unbiased) accumulates in the persistent state bank
                    nc.tensor.matmul(st[:, :], w_s["w1eps"][:, :], ep_sl,
                                     start=(t == 0), stop=(t == T - 1),
                                     skip_group_check=True)
                elif s == 1:
                    d["r1"] = sb.tile([H, CN], F32, name=f"r1_{c}", tag=f"r1_{c}")
                    nc.scalar.activation(d["r1"][:, :], st[:, :], Relu, bias=b_s["wb1"][:, :])

                elif s == 2:
                    d["a1"] = sb.tile([H, CN], BF16, name=f"a1_{c}", tag=f"a1_{c}")
                    nc.vector.tensor_tensor(d["a1"][:, :], d["r1"][:, :], d["r1"][:, :], MULT)
                elif s == 3:
                    # Batched stress rides the z2-wait bubble: the z2 matmul waits
                    # ~770ns for r1->a1, so a stress matmul placed BEFORE it in the
                    # PE FIFO executes for free.  Its PSUM->SBUF copy is deferred
                    # two steps so it never waits on the matmul (Scalar has slack).
                    if t % 2 == 1 and t >= 3:
                        if stress_pend[c] is not None:
                            ps_old, scol_old = stress_pend[c]
                            nc.scalar.activation(stg[:, scol_old:scol_old + 256],
                                                 ps_old[:, :],
                                                 mybir.ActivationFunctionType.Copy)
                            stress_pend[c] = None
                        scol = (t - 3) * 256 + c * 256
                        ps_str = strps.tile([6, 256], F32, name=f"str{c}", tag=f"str{c}")
                        nc.tensor.matmul(ps_str[:, :], w_s["w1out"][:, :],
                                         G[:, scol:scol + 256], start=True, stop=True)
                        stress_pend[c] = (ps_str, scol)
                    d["ps_z2"] = wkps.tile([H, CN], F32, name=f"psz2_{c}", tag=f"wk_{c}")
                    nc.tensor.matmul(d["ps_z2"][:, :], w_s["w2"][:, :], d["a1"][:, :],
                                     start=True, stop=True)
                elif s == 4:
                    d["r2"] = sb.tile([H, CN], BF16, name=f"r2_{c}", tag=f"r2_{c}")
                    nc.scalar.activation(d["r2"][:, :], d["ps_z2"][:, :], Relu, bias=b_s["wb2"][:, :])
                elif s == 5:
                    d["ps_g1"] = wkps.tile([H, CN], F32, name=f"psg1_{c}", tag=f"wk_{c}")
                    nc.tensor.matmul(d["ps_g1"][:, :], w_s["w2bwd"][:, :], d["r2"][:, :],
                                     start=True, stop=True)
                elif s == 6:
                    gcol = t * 256 + CN * c
                    d["g1"] = G[:, gcol:gcol + CN]
                    nc.vector.tensor_tensor(d["g1"], d["ps_g1"][:, :], d["r1"][:, :], MULT)
                elif s == 7:
                    d["ps_u1"] = wkps.tile([H, CN], F32, name=f"psu1_{c}", tag=f"wk_{c}")
                    nc.tensor.matmul(d["ps_u1"][:, :], w_s["m1"][:, :], d["g1"],
                                     start=True, stop=True)
                elif s == 8:
                    d["s1"] = sb.tile([H, CN], F32, name=f"s1_{c}", tag=f"s1_{c}")
                    nc.scalar.activation(d["s1"][:, :], d["ps_u1"][:, :], Relu, bias=b_s["db1"][:, :])
                elif s == 9:
                    d["b1a"] = sb.tile([H, CN], BF16, name=f"b1a_{c}", tag=f"b1a_{c}")
                    nc.vector.tensor_tensor(d["b1a"][:, :], d["s1"][:, :], d["s1"][:, :], MULT)
                elif s == 10:
                    d["ps_u2"] = wkps.tile([H, CN], F32, name=f"psu2_{c}", tag=f"wk_{c}")
                    nc.tensor.matmul(d["ps_u2"][:, :], w_s["dw2"][:, :], d["b1a"][:, :],
                                     start=True, stop=True)
                elif s == 11:
                    d["s2"] = sb.tile([H, CN], BF16, name=f"s2_{c}", tag=f"s2_{c}")
                    nc.scalar.activation(d["s2"][:, :], d["ps_u2"][:, :], Relu, bias=b_s["db2"][:, :])
                elif s == 12:
                    d["ps_h1"] = wkps.tile([H, CN], F32, name=f"psh1_{c}", tag=f"wk_{c}")
                    nc.tensor.matmul(d["ps_h1"][:, :], w_s["d2bwd"][:, :], d["s2"][:, :],
                                     start=True, stop=True)
                elif s == 13:
                    d["h1"] = sb.tile([H, CN], BF16, name=f"h1_{c}", tag=f"h1_{c}")
                    nc.vector.tensor_tensor(d["h1"][:, :], d["ps_h1"][:, :], d["s1"][:, :], MULT)
                elif s == 14:
                    if t < T - 1:
                        nc.tensor.matmul(st[:, :], w_s["m2t"][:, :], d["h1"][:, :],
                                         start=False, stop=False, skip_group_check=True)


            # software pipeline: chunk 1 trails chunk 0 by half a step, so each
            # engine queue's order matches the order operands become ready
            SKEW = 8
            for t in range(T):
                for i in range(NSTAGE):
                    emit_stage(t, 0, i)
                    j = i - SKEW
                    if j >= 0:
                        emit_stage(t, 1, j)
                    elif t > 0:
                        emit_stage(t - 1, 1, j + NSTAGE)
            for j in range(NSTAGE - SKEW, NSTAGE):
                emit_stage(T - 1, 1, j)
            # drain pending copies (pair 60,61) and do the last pair (62,63)
            for c in range(NCH):
                if stress_pend[c] is not None:
                    ps_old, scol_old = stress_pend[c]
                    nc.scalar.activation(stg[:, scol_old:scol_old + 256], ps_old[:, :],
                                         mybir.ActivationFunctionType.Copy)
                    stress_pend[c] = None
            for k, scol in enumerate(range((T - 2) * 256, T * 256, 256)):
                ps_str = strps.tile([6, 256], F32, name="strf", tag=f"str{k % 2}")
                nc.tensor.matmul(ps_str[:, :], w_s["w1out"][:, :],
                                 G[:, scol:scol + 256], start=True, stop=True)
                nc.scalar.activation(stg[:, scol:scol + 256], ps_str[:, :],
                                     mybir.ActivationFunctionType.Copy)

            for g in range(4):
                nc.sync.dma_start(out=out_d[:, g * 4096:(g + 1) * 4096],
                                  in_=stg[:, g * 4096:(g + 1) * 4096])

    _split_multi_waits(nc)
    return nc


def _host_prep(inputs):
    f32 = np.float32
    wW1 = np.ascontiguousarray(inputs["wW1"], f32)
    wW2 = np.ascontiguousarray(inputs["wW2"], f32)
    wW3 = np.ascontiguousarray(inputs["wW3"], f32)
    dW1 = np.ascontiguousarray(inputs["dW1"], f32)
    dW2 = np.ascontiguousarray(inputs["dW2"], f32)
    dWc = np.ascontiguousarray(inputs["dWc"], f32)
    W1eps = wW1[:6]
    W1xi = wW1[6:]
    weights = {
        "w1eps": W1eps,
        "w2": wW2,
        "w2bwd": (wW2.T * (4.0 * wW3[:, 0])[:, None]),
        "m1": -(W1xi.T @ dW1),
        "dw2": dW2,
        "d2bwd": (dW2.T * (4.0 * dWc[:, 0] ** 2)[:, None]),
        "m2t": DT * (dW1.T @ W1xi),
        "w1out": W1eps.T,
    }
    weights = {n: np.ascontiguousarray(w.astype(f32).astype(BF)) for n, w in weights.items()}
    for n in _BIAS_NAMES:
        weights[n] = np.ascontiguousarray(inputs[n], f32).reshape(H, 1)
    return weights


def _pack_deps_all(eps):
    """eps [B,T,6] -> per-core delta-eps staging [NCORES][6, T*NPC] in bf16,
    quantized with error feedback so the cumsum of quantized deltas tracks
    (eps_t - eye) to within one bf16 ulp (no error accumulation in the
    recurrent state)."""
    eye = np.array([1.0, 0.0, 0.0, 1.0, 0.0, 1.0], np.float32)
    epsT = np.ascontiguousarray(eps.transpose(1, 2, 0))  # [T, 6, B]
    tgt = epsT.astype(np.float64)
    tgt -= eye[None, :, None]
    qd = np.zeros(epsT.shape, BF)
    run = np.zeros(epsT.shape[1:], np.float64)
    for t in range(T):
        qd[t] = (tgt[t] - run).astype(np.float32).astype(BF)
        run += qd[t].astype(np.float64)
    out = []
    for core in range(NCORES):
        blk = qd[:, :, core * NPC:(core + 1) * NPC]       # [T, 6, NPC]
        out.append(np.ascontiguousarray(blk.transpose(1, 0, 2).reshape(6, T * NPC)))
    return out


def _unpack_stress(S):
    """staging [6, T*256] -> [NPC, T, 6]."""
    return np.ascontiguousarray(S.reshape(6, T, NPC).transpose(2, 1, 0))


def kernel(**inputs):
    global _CACHED_NC
    if _CACHED_NC is None:
        _CACHED_NC = _build()
    nc = _CACHED_NC

    weights = _host_prep(inputs)
    eps = np.ascontiguousarray(inputs["eps"], np.float32)
    deps_cores = _pack_deps_all(eps)
    in_maps = []
    for core in range(NCORES):
        m = dict(weights)
        m["deps"] = deps_cores[core]
        in_maps.append(m)

    res = run_bass_kernel_spmd(nc, in_maps, core_ids=list(range(NCORES)))
    out = np.empty((B, T, 6), np.float32)
    for core in range(NCORES):
        out[core * NPC:(core + 1) * NPC] = _unpack_stress(res.results[core]["stress"])
    return out
